# revision 4
# baseline (speedup 1.0000x reference)
"""Causal multi-head attention on 8 Trainium2 NeuronCores.

Problem: B=4, T=2048, C=1024, H=16 heads, D=64, fp32.
Sharding: 4-way data parallel on batch x 2-way tensor parallel on heads.
Core c -> batch c//2, heads (c%2)*8 .. (c%2)*8+7.

Per-core dataflow (bf16 matmul inputs, fp32 PSUM accumulation):
  QT(d,t) = wqT.T @ xT          (d on partitions, 2 heads per 128-row tile)
  KT(d,t) likewise; V(t,d) with an appended ones column.
  ST(k,q) = KT_h.T @ QT_h per 128-key tile (K=64 contraction -> two heads
            packed into PE row groups 0-63 / 64-127, running concurrently)
  PT = exp(ST/8)  on ScalarE (scores are ~N(0,1): no max subtraction needed)
  causal mask on diagonal tiles only, via gpsimd affine_select
  [outT; l] = [V|1].T @ PT  accumulated over key tiles (row 64 = softmax sum)
  attnT = outT * broadcast(1/l)   (K=1 float32r matmul broadcasts 1/l)
  yT(o,t) = woT.T @ attnT   -> partial output, host sums the 2 TP cores.
"""

import numpy as np
import ml_dtypes

B, T, C = 4, 2048, 1024
H, D = 16, 64
HL = 8           # local heads per core
DL = HL * D      # 512 local channels
N_CORES = 8
QB = 512         # query block (matmul moving dim)
KT_TILE = 128    # key tile (contraction tile for attn out)
NQB = T // QB    # 4 query blocks
NCT = C // 128   # 8 contraction tiles over C
BF16 = ml_dtypes.bfloat16

_CACHE: dict = {}


def _build_nc():
    import concourse.bass as bass
    from concourse import bacc, mybir, tile

    f32 = mybir.dt.float32
    f32r = mybir.dt.float32r
    bf16 = mybir.dt.bfloat16
    EXP = mybir.ActivationFunctionType.Exp

    nc = bacc.Bacc("TRN2", target_bir_lowering=False, debug=False)

    xT_d = nc.dram_tensor("xt", [C, T], bf16, kind="ExternalInput").ap()
    wq_d = nc.dram_tensor("wqt", [C, DL], bf16, kind="ExternalInput").ap()
    wk_d = nc.dram_tensor("wkt", [C, DL], bf16, kind="ExternalInput").ap()
    wv_d = nc.dram_tensor("wvt", [C, DL], bf16, kind="ExternalInput").ap()
    wo_d = nc.dram_tensor("wot", [DL, C], bf16, kind="ExternalInput").ap()
    yT_d = nc.dram_tensor("yt", [C, T], f32, kind="ExternalOutput").ap()

    with tile.TileContext(nc) as tc:
        with (
            tc.tile_pool(name="const", bufs=1) as const,
            tc.tile_pool(name="ps", bufs=4, space="PSUM") as ps_pool,
            tc.tile_pool(name="ot", bufs=4, space="PSUM") as ot_pool,
            tc.tile_pool(name="pt", bufs=6) as pt_pool,
            tc.tile_pool(name="small", bufs=4) as small,
            tc.tile_pool(name="ystage", bufs=4) as ystage,
        ):
            xT_sb = const.tile([128, NCT, T], bf16)
            wq_sb = const.tile([128, NCT, DL], bf16)
            wk_sb = const.tile([128, NCT, DL], bf16)
            wv_sb = const.tile([128, NCT, DL], bf16)
            wo_sb = const.tile([128, DL // 128, C], bf16)
            QT_sb = const.tile([128, HL // 2, T], bf16)
            KT_sb = const.tile([128, HL // 2, T], bf16)
            V_sb = const.tile([128, T // KT_TILE, HL, D + 1], bf16)
            AT_sb = const.tile([128, HL // 2, T], bf16)
            ones_sb = const.tile([1, D], bf16)

            nc.vector.memset(ones_sb[:], 1.0)
            nc.vector.memset(V_sb[:, :, :, D : D + 1], 1.0)

            for c in range(NCT):
                nc.sync.dma_start(xT_sb[:, c, :], xT_d[c * 128 : (c + 1) * 128, :])
                nc.sync.dma_start(wq_sb[:, c, :], wq_d[c * 128 : (c + 1) * 128, :])
                nc.sync.dma_start(wk_sb[:, c, :], wk_d[c * 128 : (c + 1) * 128, :])
                nc.sync.dma_start(wv_sb[:, c, :], wv_d[c * 128 : (c + 1) * 128, :])
            for r in range(DL // 128):
                nc.sync.dma_start(wo_sb[:, r, :], wo_d[r * 128 : (r + 1) * 128, :])

            # ---- Phase 1: projections ----
            # QT/KT: (dl, t) with dl tiled by 128 (= head pair j)
            for w_sb, out_sb in ((wq_sb, QT_sb), (wk_sb, KT_sb)):
                for j in range(HL // 2):
                    for tb in range(NQB):
                        acc = ps_pool.tile([128, QB], f32, tag="ps")
                        for c in range(NCT):
                            nc.tensor.matmul(
                                acc[:],
                                lhsT=w_sb[:, c, j * 128 : (j + 1) * 128],
                                rhs=xT_sb[:, c, tb * QB : (tb + 1) * QB],
                                start=(c == 0),
                                stop=(c == NCT - 1),
                            )
                        nc.vector.tensor_copy(
                            out_sb[:, j, tb * QB : (tb + 1) * QB], acc[:]
                        )
            # V natural: (t, dl) with t tiled by 128
            for tt in range(T // 128):
                acc = ps_pool.tile([128, DL], f32, tag="ps")
                for c in range(NCT):
                    nc.tensor.matmul(
                        acc[:],
                        lhsT=xT_sb[:, c, tt * 128 : (tt + 1) * 128],
                        rhs=wv_sb[:, c, :],
                        start=(c == 0),
                        stop=(c == NCT - 1),
                    )
                nc.vector.tensor_copy(
                    V_sb[:, tt, :, 0:D],
                    acc.rearrange("p (h d) -> p h d", h=HL),
                )

            # ---- Phase 2+3: attention per query block, then out-projection ----
            for qb in range(NQB):
                q0 = qb * QB
                kb = 4 * (qb + 1)  # causal: key tiles 0..kb-1 reach this q block
                for j in range(HL // 2):
                    h0, h1 = 2 * j, 2 * j + 1
                    ot0 = ot_pool.tile([D + 1, QB], f32, tag="ot")
                    ot1 = ot_pool.tile([D + 1, QB], f32, tag="ot")
                    for k in range(kb):
                        k0 = k * KT_TILE
                        pts = []
                        for hh, base in ((h0, 0), (h1, 64)):
                            st = ps_pool.tile([128, QB], f32, tag="ps")
                            nc.tensor.matmul(
                                st[:],
                                lhsT=KT_sb[base : base + 64, j, k0 : k0 + 128],
                                rhs=QT_sb[base : base + 64, j, q0 : q0 + QB],
                                start=True,
                                stop=True,
                            )
                            pt = pt_pool.tile([128, QB], bf16, tag="pt")
                            # P = exp(S / sqrt(D)); scores are O(1) so no
                            # max-subtraction is needed for fp32/bf16 range.
                            nc.scalar.activation(pt[:], st[:], EXP, scale=0.125)
                            if k0 > q0 - 128:
                                # diagonal tile: zero where key > query
                                nc.gpsimd.affine_select(
                                    out=pt[:],
                                    in_=pt[:],
                                    pattern=[[1, QB]],
                                    compare_op=mybir.AluOpType.is_ge,
                                    fill=0.0,
                                    base=q0 - k0,
                                    channel_multiplier=-1,
                                )
                            pts.append(pt)
                        for ot, pt, hh in ((ot0, pts[0], h0), (ot1, pts[1], h1)):
                            nc.tensor.matmul(
                                ot[:],
                                lhsT=V_sb[:, k, hh, :],
                                rhs=pt[:],
                                start=(k == 0),
                                stop=(k == kb - 1),
                            )
                    for ot, hh in ((ot0, h0), (ot1, h1)):
                        rT = small.tile([1, QB], bf16, tag="rT")
                        with nc.allow_low_precision(
                            reason="1/l broadcast via bf16 matmul; ~2^-9 rel"
                        ):
                            nc.vector.reciprocal(rT[:], ot[D : D + 1, :])
                        bc = ps_pool.tile([D, QB], f32, tag="ps")
                        nc.tensor.matmul(
                            bc[:],
                            lhsT=ones_sb[:],
                            rhs=rT[:],
                            start=True,
                            stop=True,
                        )
                        bcs = small.tile([D, QB], f32, tag="bcs")
                        nc.vector.tensor_copy(bcs[:], bc[:])
                        base = 64 * (hh % 2)
                        nc.vector.tensor_mul(
                            AT_sb[base : base + D, hh // 2, q0 : q0 + QB],
                            ot[0:D, :],
                            bcs[:],
                        )
                # out projection for this query block
                for ob in range(C // 128):
                    acc = ps_pool.tile([128, QB], f32, tag="ps")
                    for r in range(DL // 128):
                        nc.tensor.matmul(
                            acc[:],
                            lhsT=wo_sb[:, r, ob * 128 : (ob + 1) * 128],
                            rhs=AT_sb[:, r, q0 : q0 + QB],
                            start=(r == 0),
                            stop=(r == DL // 128 - 1),
                        )
                    yst = ystage.tile([128, QB], f32, tag="yst")
                    nc.vector.tensor_copy(yst[:], acc[:])
                    nc.sync.dma_start(
                        yT_d[ob * 128 : (ob + 1) * 128, q0 : q0 + QB], yst[:]
                    )

    nc.compile()
    return nc


def _get_nc():
    if "nc" not in _CACHE:
        _CACHE["nc"] = _build_nc()
    return _CACHE["nc"]


def _run(in_maps, trace=False):
    from concourse.bass_utils import run_bass_kernel_spmd

    nc = _get_nc()
    return run_bass_kernel_spmd(nc, in_maps, list(range(N_CORES)), trace=trace)


def _make_in_maps(x, W_Q, W_K, W_V, W_out):
    x = np.asarray(x, dtype=np.float32)
    W_Q = np.asarray(W_Q, dtype=np.float32)
    W_K = np.asarray(W_K, dtype=np.float32)
    W_V = np.asarray(W_V, dtype=np.float32)
    W_out = np.asarray(W_out, dtype=np.float32)

    in_maps = []
    for core in range(N_CORES):
        b, hh = core // 2, core % 2
        sl = slice(hh * DL, (hh + 1) * DL)
        in_maps.append(
            {
                "xt": np.ascontiguousarray(x[b].T).astype(BF16),
                "wqt": np.ascontiguousarray(W_Q[sl, :].T).astype(BF16),
                "wkt": np.ascontiguousarray(W_K[sl, :].T).astype(BF16),
                "wvt": np.ascontiguousarray(W_V[sl, :].T).astype(BF16),
                "wot": np.ascontiguousarray(W_out[:, sl].T).astype(BF16),
            }
        )
    return in_maps


def _assemble(results):
    y = np.empty((B, T, C), dtype=np.float32)
    for b in range(B):
        yT = results[2 * b]["yt"] + results[2 * b + 1]["yt"]
        y[b] = yT.T
    return y


def kernel(x, W_Q, W_K, W_V, W_out):
    res = _run(_make_in_maps(x, W_Q, W_K, W_V, W_out), trace=False)
    return _assemble(res.results)


# revision 35
# speedup vs baseline: 1.3798x; 1.3798x over previous
"""Causal multi-head attention on 8 Trainium2 NeuronCores.

Problem: B=4, T=2048, C=1024, H=16 heads, D=64, fp32.
Sharding: 4-way data parallel on batch x 2-way tensor parallel on heads.
Core c -> batch c//2, heads (c%2)*8 .. (c%2)*8+7.

Per-core dataflow (bf16 matmul inputs, fp32 PSUM accumulation):
  QT(d,t) = wqT.T @ xT          (d on partitions, 2 heads per 128-row tile)
  KT(d,t) likewise; V(t,d) with an appended ones column.
  ST(k,q) = KT_h.T @ QT_h per 128-key tile (K=64 contraction -> the two
            heads of a pair go to PE row groups 0-63 / 64-127 and run
            concurrently; both land in one 2-bank PSUM tile)
  PT = exp(ST/8) on ScalarE, one double-width activation per key tile
       (scores are ~N(0,1): no max-subtraction needed)
  causal mask on diagonal tiles only, via gpsimd affine_select; diagonal
       tiles are narrowed to the unmasked query range
  [outT; l] = [V|1].T @ PT  accumulated over key tiles (row 64 = softmax sum)
  attnT = outT * bcast(1/l)  (1/l broadcast across partitions by a DMA)
  yT(o,t) = woT.T @ attnT   -> partial output, host sums the 2 TP cores.

Projections for head pair j+1 are emitted after attention of pair j so the
PE fills attention's ACT-bound stretches with projection matmuls.
"""

import numpy as np
import ml_dtypes

B, T, C = 4, 2048, 1024
H, D = 16, 64
HL = 8           # local heads per core
DL = HL * D      # 512 local channels
N_CORES = 8
QB = 512         # query block (matmul moving dim)
NQB = T // QB    # 4 query blocks
NCT = C // 128   # 8 contraction tiles over C
NJ = HL // 2     # 4 head pairs
BF16 = ml_dtypes.bfloat16

_CACHE: dict = {}


def _build_nc():
    import concourse.bass as bass
    from concourse import bacc, mybir, tile

    f32 = mybir.dt.float32
    bf16 = mybir.dt.bfloat16
    EXP = mybir.ActivationFunctionType.Exp

    nc = bacc.Bacc("TRN2", target_bir_lowering=False, debug=False)

    xT_d = nc.dram_tensor("xt", [C, T], bf16, kind="ExternalInput").ap()
    wq_d = nc.dram_tensor("wqt", [C, DL], bf16, kind="ExternalInput").ap()
    wk_d = nc.dram_tensor("wkt", [C, DL], bf16, kind="ExternalInput").ap()
    wv_d = nc.dram_tensor("wvt", [C, DL], bf16, kind="ExternalInput").ap()
    wo_d = nc.dram_tensor("wot", [DL, C], bf16, kind="ExternalInput").ap()
    yT_d = nc.dram_tensor("yt", [C, T], f32, kind="ExternalOutput").ap()

    with tile.TileContext(nc) as tc:
        with (
            tc.tile_pool(name="const", bufs=1) as const,
            tc.tile_pool(name="ps", bufs=2, space="PSUM") as ps_pool,
            tc.tile_pool(name="ot", bufs=2, space="PSUM") as ot_pool,
            tc.tile_pool(name="pt", bufs=6) as pt_pool,
            tc.tile_pool(name="small", bufs=4) as small,
            tc.tile_pool(name="ystage", bufs=4) as ystage,
            tc.tile_pool(name="dram", bufs=4, space="DRAM") as dram_pool,
        ):
            xT_sb = const.tile([128, NCT, T], bf16)
            wq_sb = const.tile([128, NCT, DL], bf16)
            wk_sb = const.tile([128, NCT, DL], bf16)
            wv_sb = const.tile([128, NCT, DL], bf16)
            wo_sb = const.tile([128, DL // 128, C], bf16)
            QT_sb = const.tile([128, NJ, T], bf16)
            KT_sb = const.tile([128, NJ, T], bf16)
            V_sb = const.tile([128, T // 128, HL, D + 1], bf16)
            AT_sb = const.tile([128, NJ, T], bf16)
            ones_sb = const.tile([1, D], bf16)

            nc.vector.memset(ones_sb[:], 1.0)
            nc.vector.memset(V_sb[:, :, :, D : D + 1], 1.0)

            # input loads: the working set of attention(0,0) first (t-block 0
            # of xT, first-half K/Q weights, all of wv), spread over several
            # issuing engines so the DGE queues run in parallel
            HDL = DL // 2
            for c in range(NCT):
                cs = slice(c * 128, (c + 1) * 128)
                nc.sync.dma_start(xT_sb[:, c, 0:QB], xT_d[cs, 0:QB])
                nc.scalar.dma_start(wk_sb[:, c, 0:HDL], wk_d[cs, 0:HDL])
                nc.gpsimd.dma_start(wq_sb[:, c, 0:HDL], wq_d[cs, 0:HDL])
            for c in range(NCT):
                nc.gpsimd.dma_start(wv_sb[:, c, :], wv_d[c * 128 : (c + 1) * 128, :])
            for tb in range(1, NQB):
                ts_ = slice(tb * QB, (tb + 1) * QB)
                for c in range(NCT):
                    nc.sync.dma_start(
                        xT_sb[:, c, ts_], xT_d[c * 128 : (c + 1) * 128, ts_]
                    )
            for c in range(NCT):
                cs = slice(c * 128, (c + 1) * 128)
                nc.scalar.dma_start(wk_sb[:, c, HDL:DL], wk_d[cs, HDL:DL])
                nc.gpsimd.dma_start(wq_sb[:, c, HDL:DL], wq_d[cs, HDL:DL])
            for r in range(DL // 128):
                nc.scalar.dma_start(wo_sb[:, r, :], wo_d[r * 128 : (r + 1) * 128, :])

            def proj_qk_block(w_sb, out_sb, j, tb):
                # (dl, t) projection for head pair j, one 512-col t block
                acc = ps_pool.tile([128, QB], f32, tag="ps")
                for c in range(NCT):
                    nc.tensor.matmul(
                        acc[:],
                        lhsT=w_sb[:, c, j * 128 : (j + 1) * 128],
                        rhs=xT_sb[:, c, tb * QB : (tb + 1) * QB],
                        start=(c == 0),
                        stop=(c == NCT - 1),
                    )
                    yield
                nc.vector.tensor_copy(out_sb[:, j, tb * QB : (tb + 1) * QB], acc[:])

            def proj_v_block(tt):
                # V natural: (t, dl) for one 128-row t tile, all heads
                acc = ps_pool.tile([128, DL], f32, tag="ps")
                for c in range(NCT):
                    nc.tensor.matmul(
                        acc[:],
                        lhsT=xT_sb[:, c, tt * 128 : (tt + 1) * 128],
                        rhs=wv_sb[:, c, :],
                        start=(c == 0),
                        stop=(c == NCT - 1),
                    )
                    yield
                nc.vector.tensor_copy(
                    V_sb[:, tt, :, 0:D],
                    acc.rearrange("p (h d) -> p h d", h=HL),
                )

            def proj_y_block(qb, ob):
                q0 = qb * QB
                acc = ps_pool.tile([128, QB], f32, tag="ps")
                for r in range(DL // 128):
                    nc.tensor.matmul(
                        acc[:],
                        lhsT=wo_sb[:, r, ob * 128 : (ob + 1) * 128],
                        rhs=AT_sb[:, r, q0 : q0 + QB],
                        start=(r == 0),
                        stop=(r == DL // 128 - 1),
                    )
                    yield
                yst = ystage.tile([128, QB], f32, tag="yst")
                nc.vector.tensor_copy(yst[:], acc[:])
                nc.sync.dma_start(
                    yT_d[ob * 128 : (ob + 1) * 128, q0 : q0 + QB], yst[:]
                )

            # ---- filler machinery: a queue of (name, generator) projection
            # blocks streamed into the attention k-loop as PE gap filler ----
            filler: dict = {"items": [], "idx": 0, "done": set()}

            def filler_add(name, gen):
                filler["items"].append((name, gen))

            def filler_pull(n):
                pulled = 0
                while pulled < n and filler["idx"] < len(filler["items"]):
                    name, gen = filler["items"][filler["idx"]]
                    try:
                        next(gen)
                        pulled += 1
                    except StopIteration:
                        filler["done"].add(name)
                        filler["idx"] += 1

            def filler_flush_until(names):
                while not all(n in filler["done"] for n in names):
                    if filler["idx"] >= len(filler["items"]):
                        missing = [n for n in names if n not in filler["done"]]
                        raise RuntimeError(f"filler queue exhausted: {missing}")
                    filler_pull(1)

            # normalization of the previous attention block is emitted just
            # after the next block's first score matmuls, so the PE does not
            # stall on the DVE reciprocal in between blocks
            pending_norm: list = []

            def flush_norm():
                while pending_norm:
                    pending_norm.pop(0)()

            def attention(j, qb):
                q0 = qb * QB
                kb = (qb + 1) * (QB // 128)  # causal reach in 128-key tiles
                h0, h1 = 2 * j, 2 * j + 1
                ot0 = ot_pool.tile([D + 1, QB], f32, tag="ot")
                ot1 = ot_pool.tile([D + 1, QB], f32, tag="ot")

                def emit_st(k):
                    k0 = k * 128
                    # diagonal tiles: only queries >= k0 are unmasked
                    w0 = max(0, k0 - q0)  # first valid query column
                    st = ps_pool.tile([128, 2, QB], f32, tag="st")
                    for hi, base in ((0, 0), (1, 64)):
                        nc.tensor.matmul(
                            st[:, hi, w0:QB],
                            lhsT=KT_sb[base : base + 64, j, k0 : k0 + 128],
                            rhs=QT_sb[base : base + 64, j, q0 + w0 : q0 + QB],
                            start=True,
                            stop=True,
                        )
                    pt = pt_pool.tile([128, 2, QB], bf16, tag="pt")
                    # P = exp(S / sqrt(D)); scores are O(1) so skipping the
                    # max-subtraction is safe in fp32/bf16 range.
                    nc.scalar.activation(
                        pt[:, :, w0:QB], st[:, :, w0:QB], EXP, scale=0.125
                    )
                    if k0 >= q0:
                        # tile crosses the causal diagonal: zero key > query
                        nc.gpsimd.affine_select(
                            out=pt[:, :, w0:QB],
                            in_=pt[:, :, w0:QB],
                            pattern=[[0, 2], [1, QB - w0]],
                            compare_op=mybir.AluOpType.is_ge,
                            fill=0.0,
                            base=q0 + w0 - k0,
                            channel_multiplier=-1,
                        )
                    return pt, w0

                def emit_ot(k, pt, w0):
                    for ot, hi, hh in ((ot0, 0, h0), (ot1, 1, h1)):
                        nc.tensor.matmul(
                            ot[:, w0:QB],
                            lhsT=V_sb[:, k, hh, :],
                            rhs=pt[:, hi, w0:QB],
                            start=(k == 0),
                            stop=(k == kb - 1),
                        )

                # software pipeline: PE issues st[k+1] before ot[k] so the
                # exp of st[k] overlaps PE work instead of stalling it;
                # projection fillers pad each slot up to the exp latency
                prev = emit_st(0)
                flush_norm()
                for k in range(1, kb):
                    cur = emit_st(k)
                    filler_pull(2)
                    emit_ot(k - 1, *prev)
                    prev = cur
                filler_pull(2)
                emit_ot(kb - 1, *prev)

                # short-latency path everywhere: the ot PSUM slots (bufs=2)
                # are only released once the norm mul has read them, so a
                # long norm chain stalls the next block's accumulation
                fast = True

                def norm(ot=ot0, hh=h0, ot_b=ot1, hh_b=h1, q0=q0, fast=fast):
                    for o, h in ((ot, hh), (ot_b, hh_b)):
                        bcs = small.tile([D, QB], f32, tag="bcs", name="bcs")
                        if fast:
                            # broadcast 1/l via K=1 matmul (short latency)
                            rT = small.tile([1, QB], bf16, tag="rT", name="rT")
                            with nc.allow_low_precision(reason="bf16 1/l bcast"):
                                nc.vector.reciprocal(rT[:], o[D : D + 1, :])
                            bc = ps_pool.tile([D, QB], f32, tag="ps", name="bc")
                            nc.tensor.matmul(
                                bc[:], lhsT=ones_sb[:], rhs=rT[:],
                                start=True, stop=True,
                            )
                            nc.vector.tensor_copy(bcs[:], bc[:])
                        else:
                            # broadcast 1/l across partitions by bouncing
                            # through DRAM (DRAM-source DMAs may replicate);
                            # off the PE, latency hidden by later blocks
                            rT = small.tile([1, QB], f32, tag="rT", name="rT")
                            nc.vector.reciprocal(rT[:], o[D : D + 1, :])
                            rd = dram_pool.tile([QB], f32, name="rd")
                            nc.scalar.dma_start(rd[:], rT[0, :])
                            import concourse.bass as bass_mod

                            r_bcast = bass_mod.AP(
                                tensor=rd.tensor,
                                offset=rd.offset,
                                ap=[[0, D], [1, QB]],
                            )
                            nc.scalar.dma_start(bcs[:], r_bcast)
                        base = 64 * (h % 2)
                        nc.vector.tensor_mul(
                            AT_sb[base : base + D, h // 2, q0 : q0 + QB],
                            o[0:D, :],
                            bcs[:],
                        )

                pending_norm.append(norm)

            def run(gen):
                for _ in gen:
                    pass

            # Build the filler queue: everything except the j=0/qb=0
            # prerequisites, in rough just-in-time order.
            for qb in range(1, NQB):
                filler_add(f"kq0.{qb}k", proj_qk_block(wk_sb, KT_sb, 0, qb))
                filler_add(f"kq0.{qb}q", proj_qk_block(wq_sb, QT_sb, 0, qb))
                for tt in range(4 * qb, 4 * qb + 4):
                    filler_add(f"v{tt}", proj_v_block(tt))
            for j in range(1, NJ):
                for qb in range(NQB):
                    filler_add(f"kq{j}.{qb}k", proj_qk_block(wk_sb, KT_sb, j, qb))
                    filler_add(f"kq{j}.{qb}q", proj_qk_block(wq_sb, QT_sb, j, qb))
            # y blocks are appended only after the attention that writes
            # their AT_sb input has been emitted (program-order correctness)

            def need_attention(j, qb):
                if j == 0:
                    if qb == 0:
                        return []
                    names = [f"kq0.{t}k" for t in range(1, qb + 1)]
                    names += [f"kq0.{qb}q"]
                    names += [f"v{t}" for t in range(4, 4 * qb + 4)]
                    return names
                names = [f"kq{j}.{t}k" for t in range(qb + 1)]
                names += [f"kq{j}.{qb}q"]
                return names

            # j=0/qb=0 prerequisites emitted directly
            run(proj_qk_block(wk_sb, KT_sb, 0, 0))
            run(proj_qk_block(wq_sb, QT_sb, 0, 0))
            for tt in range(4):
                run(proj_v_block(tt))

            for j in range(NJ):
                for qb in range(NQB):
                    filler_flush_until(need_attention(j, qb))
                    attention(j, qb)
                    if j == NJ - 1:
                        for ob in range(C // 128):
                            filler_add(f"y{qb}.{ob}", proj_y_block(qb, ob))
            # drain the last norm and remaining fillers (tail y projections)
            flush_norm()
            filler_pull(1_000_000_000)

    nc.compile()
    return nc


def _get_nc():
    if "nc" not in _CACHE:
        _CACHE["nc"] = _build_nc()
    return _CACHE["nc"]


def _run(in_maps, trace=False):
    from concourse.bass_utils import run_bass_kernel_spmd

    nc = _get_nc()
    return run_bass_kernel_spmd(nc, in_maps, list(range(N_CORES)), trace=trace)


def _make_in_maps(x, W_Q, W_K, W_V, W_out):
    x = np.asarray(x, dtype=np.float32)
    W_Q = np.asarray(W_Q, dtype=np.float32)
    W_K = np.asarray(W_K, dtype=np.float32)
    W_V = np.asarray(W_V, dtype=np.float32)
    W_out = np.asarray(W_out, dtype=np.float32)

    in_maps = []
    for core in range(N_CORES):
        b, hh = core // 2, core % 2
        sl = slice(hh * DL, (hh + 1) * DL)
        in_maps.append(
            {
                "xt": np.ascontiguousarray(x[b].T).astype(BF16),
                "wqt": np.ascontiguousarray(W_Q[sl, :].T).astype(BF16),
                "wkt": np.ascontiguousarray(W_K[sl, :].T).astype(BF16),
                "wvt": np.ascontiguousarray(W_V[sl, :].T).astype(BF16),
                "wot": np.ascontiguousarray(W_out[:, sl].T).astype(BF16),
            }
        )
    return in_maps


def _assemble(results):
    y = np.empty((B, T, C), dtype=np.float32)
    for b in range(B):
        yT = results[2 * b]["yt"] + results[2 * b + 1]["yt"]
        y[b] = yT.T
    return y


def kernel(x, W_Q, W_K, W_V, W_out):
    res = _run(_make_in_maps(x, W_Q, W_K, W_V, W_out), trace=False)
    return _assemble(res.results)


# revision 43
# speedup vs baseline: 1.4975x; 1.0853x over previous
"""Causal multi-head attention on 8 Trainium2 NeuronCores.

Problem: B=4, T=2048, C=1024, H=16 heads, D=64, fp32.
Sharding: 4-way data parallel on batch x 2-way tensor parallel on heads.
Core c -> batch c//2, heads (c%2)*8 .. (c%2)*8+7.

Per-core dataflow (bf16 matmul inputs, fp32 PSUM accumulation):
  QT(d,t) = wqT.T @ xT          (d on partitions, 2 heads per 128-row tile)
  KT(d,t) likewise; V(t,d) with an appended ones column.
  ST(k,q) = KT_h.T @ QT_h per 128-key tile (K=64 contraction -> the two
            heads of a pair go to PE row groups 0-63 / 64-127 and run
            concurrently; both land in one 2-bank PSUM tile)
  PT = exp(ST/8) on ScalarE, one double-width activation per key tile
       (scores are ~N(0,1): no max-subtraction needed)
  causal mask on diagonal tiles only, via gpsimd affine_select; diagonal
       tiles are narrowed to the unmasked query range
  [outT; l] = [V|1].T @ PT  accumulated over key tiles (row 64 = softmax sum)
  attnT = outT * bcast(1/l)  (1/l broadcast across partitions by a DMA)
  yT(o,t) = woT.T @ attnT   -> partial output, host sums the 2 TP cores.

Projections for head pair j+1 are emitted after attention of pair j so the
PE fills attention's ACT-bound stretches with projection matmuls.
"""

import numpy as np
import ml_dtypes

B, T, C = 4, 2048, 1024
H, D = 16, 64
HL = 8           # local heads per core
DL = HL * D      # 512 local channels
N_CORES = 8
QB = 512         # query block (matmul moving dim)
NQB = T // QB    # 4 query blocks
NCT = C // 128   # 8 contraction tiles over C
NJ = HL // 2     # 4 head pairs
BF16 = ml_dtypes.bfloat16

_CACHE: dict = {}


def _build_nc(repeat=1):
    import contextlib

    import concourse.bass as bass
    from concourse import bacc, mybir, tile

    f32 = mybir.dt.float32
    bf16 = mybir.dt.bfloat16
    EXP = mybir.ActivationFunctionType.Exp

    nc = bacc.Bacc("TRN2", target_bir_lowering=False, debug=False)

    xT_d = nc.dram_tensor("xt", [C, T], bf16, kind="ExternalInput").ap()
    wq_d = nc.dram_tensor("wqt", [C, DL], bf16, kind="ExternalInput").ap()
    wk_d = nc.dram_tensor("wkt", [C, DL], bf16, kind="ExternalInput").ap()
    wv_d = nc.dram_tensor("wvt", [C, DL], bf16, kind="ExternalInput").ap()
    wo_d = nc.dram_tensor("wot", [DL, C], bf16, kind="ExternalInput").ap()
    yT_d = nc.dram_tensor("yt", [C, T], bf16, kind="ExternalOutput").ap()

    with tile.TileContext(nc) as tc:
        with (
            tc.tile_pool(name="const", bufs=1) as const,
            tc.tile_pool(name="ps", bufs=2, space="PSUM") as ps_pool,
            tc.tile_pool(name="ot", bufs=2, space="PSUM") as ot_pool,
            tc.tile_pool(name="pt", bufs=8) as pt_pool,
            tc.tile_pool(name="small", bufs=4) as small,
            tc.tile_pool(name="ystage", bufs=4) as ystage,
            tc.tile_pool(name="dram", bufs=4, space="DRAM") as dram_pool,
            tc.For_i(0, repeat, 1) if repeat > 1 else contextlib.nullcontext(),
        ):
            xT_sb = const.tile([128, NCT, T], bf16)
            wq_sb = const.tile([128, NCT, DL], bf16)
            wk_sb = const.tile([128, NCT, DL], bf16)
            wv_sb = const.tile([128, NCT, DL], bf16)
            wo_sb = const.tile([128, DL // 128, C], bf16)
            QT_sb = const.tile([128, NJ, T], bf16)
            KT_sb = const.tile([128, NJ, T], bf16)
            V_sb = const.tile([128, T // 128, HL, D + 1], bf16)
            AT_sb = const.tile([128, NJ, T], bf16)
            ones_sb = const.tile([1, D], bf16)
            mask_sb = const.tile([128, QB], bf16)

            nc.vector.memset(ones_sb[:], 1.0)
            nc.vector.memset(V_sb[:, :, :, D : D + 1], 1.0)
            # causal mask for diagonal tiles, relative layout: keep f >= p.
            # Every diagonal tile uses the same pattern on its w0: slice.
            nc.vector.memset(mask_sb[:], 1.0)
            nc.gpsimd.affine_select(
                out=mask_sb[:],
                in_=mask_sb[:],
                pattern=[[1, QB]],
                compare_op=mybir.AluOpType.is_ge,
                fill=0.0,
                base=0,
                channel_multiplier=-1,
            )

            # input loads: the working set of attention(0,0) first (t-block 0
            # of xT, first-half K/Q weights, all of wv), spread over several
            # issuing engines so the DGE queues run in parallel
            HDL = DL // 2
            for c in range(NCT):
                cs = slice(c * 128, (c + 1) * 128)
                nc.sync.dma_start(xT_sb[:, c, 0:QB], xT_d[cs, 0:QB])
                nc.scalar.dma_start(wk_sb[:, c, 0:HDL], wk_d[cs, 0:HDL])
                nc.gpsimd.dma_start(wq_sb[:, c, 0:HDL], wq_d[cs, 0:HDL])
            for c in range(NCT):
                nc.gpsimd.dma_start(wv_sb[:, c, :], wv_d[c * 128 : (c + 1) * 128, :])
            for tb in range(1, NQB):
                ts_ = slice(tb * QB, (tb + 1) * QB)
                for c in range(NCT):
                    nc.sync.dma_start(
                        xT_sb[:, c, ts_], xT_d[c * 128 : (c + 1) * 128, ts_]
                    )
            for c in range(NCT):
                cs = slice(c * 128, (c + 1) * 128)
                nc.scalar.dma_start(wk_sb[:, c, HDL:DL], wk_d[cs, HDL:DL])
                nc.gpsimd.dma_start(wq_sb[:, c, HDL:DL], wq_d[cs, HDL:DL])
            for r in range(DL // 128):
                nc.scalar.dma_start(wo_sb[:, r, :], wo_d[r * 128 : (r + 1) * 128, :])

            def proj_qk_block(w_sb, out_sb, j, tb):
                # (dl, t) projection for head pair j, one 512-col t block
                acc = ps_pool.tile([128, QB], f32, tag="ps")
                for c in range(NCT):
                    nc.tensor.matmul(
                        acc[:],
                        lhsT=w_sb[:, c, j * 128 : (j + 1) * 128],
                        rhs=xT_sb[:, c, tb * QB : (tb + 1) * QB],
                        start=(c == 0),
                        stop=(c == NCT - 1),
                    )
                    yield
                nc.vector.tensor_copy(out_sb[:, j, tb * QB : (tb + 1) * QB], acc[:])

            def proj_v_block(tt):
                # V natural: (t, dl) for one 128-row t tile, all heads
                acc = ps_pool.tile([128, DL], f32, tag="ps")
                for c in range(NCT):
                    nc.tensor.matmul(
                        acc[:],
                        lhsT=xT_sb[:, c, tt * 128 : (tt + 1) * 128],
                        rhs=wv_sb[:, c, :],
                        start=(c == 0),
                        stop=(c == NCT - 1),
                    )
                    yield
                nc.vector.tensor_copy(
                    V_sb[:, tt, :, 0:D],
                    acc.rearrange("p (h d) -> p h d", h=HL),
                )

            def proj_y_block(qb, ob):
                q0 = qb * QB
                acc = ps_pool.tile([128, QB], f32, tag="ps")
                for r in range(DL // 128):
                    nc.tensor.matmul(
                        acc[:],
                        lhsT=wo_sb[:, r, ob * 128 : (ob + 1) * 128],
                        rhs=AT_sb[:, r, q0 : q0 + QB],
                        start=(r == 0),
                        stop=(r == DL // 128 - 1),
                    )
                    yield
                yst = ystage.tile([128, QB], bf16, tag="yst")
                nc.vector.tensor_copy(yst[:], acc[:])
                nc.sync.dma_start(
                    yT_d[ob * 128 : (ob + 1) * 128, q0 : q0 + QB], yst[:]
                )

            # ---- filler machinery: a queue of (name, generator) projection
            # blocks streamed into the attention k-loop as PE gap filler ----
            filler: dict = {"items": [], "idx": 0, "done": set()}

            def filler_add(name, gen):
                filler["items"].append((name, gen))

            def filler_pull(n):
                pulled = 0
                while pulled < n and filler["idx"] < len(filler["items"]):
                    name, gen = filler["items"][filler["idx"]]
                    try:
                        next(gen)
                        pulled += 1
                    except StopIteration:
                        filler["done"].add(name)
                        filler["idx"] += 1

            def filler_flush_until(names):
                while not all(n in filler["done"] for n in names):
                    if filler["idx"] >= len(filler["items"]):
                        missing = [n for n in names if n not in filler["done"]]
                        raise RuntimeError(f"filler queue exhausted: {missing}")
                    filler_pull(1)

            # normalization of the previous attention block is emitted just
            # after the next block's first score matmuls, so the PE does not
            # stall on the DVE reciprocal in between blocks
            pending_norm: list = []

            def flush_norm():
                while pending_norm:
                    pending_norm.pop(0)()

            def attention(j, qb):
                q0 = qb * QB
                kb = (qb + 1) * (QB // 128)  # causal reach in 128-key tiles
                h0, h1 = 2 * j, 2 * j + 1
                ot0 = ot_pool.tile([D + 1, QB], f32, tag="ot")
                ot1 = ot_pool.tile([D + 1, QB], f32, tag="ot")

                def emit_st(k):
                    k0 = k * 128
                    # diagonal tiles: only queries >= k0 are unmasked
                    w0 = max(0, k0 - q0)  # first valid query column
                    st = ps_pool.tile([128, 2, QB], f32, tag="st")
                    for hi, base in ((0, 0), (1, 64)):
                        nc.tensor.matmul(
                            st[:, hi, w0:QB],
                            lhsT=KT_sb[base : base + 64, j, k0 : k0 + 128],
                            rhs=QT_sb[base : base + 64, j, q0 + w0 : q0 + QB],
                            start=True,
                            stop=True,
                        )
                    pt = pt_pool.tile([128, 2, QB], bf16, tag="pt")
                    # P = exp(S / sqrt(D)); scores are O(1) so skipping the
                    # max-subtraction is safe in fp32/bf16 range.
                    nc.scalar.activation(
                        pt[:, :, w0:QB], st[:, :, w0:QB], EXP, scale=0.125
                    )
                    if k0 >= q0:
                        # tile crosses the causal diagonal: zero key > query
                        # (DVE multiply by the precomputed relative mask,
                        # broadcast over the two heads via a 0-stride dim)
                        m_ap = bass.AP(
                            tensor=mask_sb.tensor,
                            offset=mask_sb.offset,
                            ap=[mask_sb.ap[0], [0, 2], [1, QB - w0]],
                        )
                        nc.vector.tensor_mul(pt[:, :, w0:QB], pt[:, :, w0:QB], m_ap)
                    return pt, w0

                def emit_ot(k, pt, w0):
                    for ot, hi, hh in ((ot0, 0, h0), (ot1, 1, h1)):
                        nc.tensor.matmul(
                            ot[:, w0:QB],
                            lhsT=V_sb[:, k, hh, :],
                            rhs=pt[:, hi, w0:QB],
                            start=(k == 0),
                            stop=(k == kb - 1),
                        )

                # software pipeline: PE issues st[k+1] before ot[k] so the
                # exp of st[k] overlaps PE work instead of stalling it;
                # projection fillers pad each slot up to the exp latency
                prev = emit_st(0)
                flush_norm()
                for k in range(1, kb):
                    cur = emit_st(k)
                    filler_pull(2)
                    emit_ot(k - 1, *prev)
                    prev = cur
                filler_pull(2)
                emit_ot(kb - 1, *prev)

                # short-latency path everywhere: the ot PSUM slots (bufs=2)
                # are only released once the norm mul has read them, so a
                # long norm chain stalls the next block's accumulation
                fast = True

                def norm(ot=ot0, hh=h0, ot_b=ot1, hh_b=h1, q0=q0, fast=fast):
                    for o, h in ((ot, hh), (ot_b, hh_b)):
                        bcs = small.tile([D, QB], f32, tag="bcs", name="bcs")
                        if fast:
                            # broadcast 1/l via K=1 matmul (short latency)
                            rT = small.tile([1, QB], bf16, tag="rT", name="rT")
                            with nc.allow_low_precision(reason="bf16 1/l bcast"):
                                nc.vector.reciprocal(rT[:], o[D : D + 1, :])
                            bc = ps_pool.tile([D, QB], f32, tag="ps", name="bc")
                            nc.tensor.matmul(
                                bc[:], lhsT=ones_sb[:], rhs=rT[:],
                                start=True, stop=True,
                            )
                            nc.vector.tensor_copy(bcs[:], bc[:])
                        else:
                            # broadcast 1/l across partitions by bouncing
                            # through DRAM (DRAM-source DMAs may replicate);
                            # off the PE, latency hidden by later blocks
                            rT = small.tile([1, QB], f32, tag="rT", name="rT")
                            nc.vector.reciprocal(rT[:], o[D : D + 1, :])
                            rd = dram_pool.tile([QB], f32, name="rd")
                            nc.scalar.dma_start(rd[:], rT[0, :])
                            import concourse.bass as bass_mod

                            r_bcast = bass_mod.AP(
                                tensor=rd.tensor,
                                offset=rd.offset,
                                ap=[[0, D], [1, QB]],
                            )
                            nc.scalar.dma_start(bcs[:], r_bcast)
                        base = 64 * (h % 2)
                        nc.vector.tensor_mul(
                            AT_sb[base : base + D, h // 2, q0 : q0 + QB],
                            o[0:D, :],
                            bcs[:],
                        )

                pending_norm.append(norm)

            def run(gen):
                for _ in gen:
                    pass

            # Build the filler queue: everything except the j=0/qb=0
            # prerequisites, in rough just-in-time order.
            for qb in range(1, NQB):
                filler_add(f"kq0.{qb}k", proj_qk_block(wk_sb, KT_sb, 0, qb))
                filler_add(f"kq0.{qb}q", proj_qk_block(wq_sb, QT_sb, 0, qb))
                for tt in range(4 * qb, 4 * qb + 4):
                    filler_add(f"v{tt}", proj_v_block(tt))
            for j in range(1, NJ):
                for qb in range(NQB):
                    filler_add(f"kq{j}.{qb}k", proj_qk_block(wk_sb, KT_sb, j, qb))
                    filler_add(f"kq{j}.{qb}q", proj_qk_block(wq_sb, QT_sb, j, qb))
            # y blocks are appended only after the attention that writes
            # their AT_sb input has been emitted (program-order correctness)

            def need_attention(j, qb):
                if j == 0:
                    if qb == 0:
                        return []
                    names = [f"kq0.{t}k" for t in range(1, qb + 1)]
                    names += [f"kq0.{qb}q"]
                    names += [f"v{t}" for t in range(4, 4 * qb + 4)]
                    return names
                names = [f"kq{j}.{t}k" for t in range(qb + 1)]
                names += [f"kq{j}.{qb}q"]
                return names

            # j=0/qb=0 prerequisites emitted directly
            run(proj_qk_block(wk_sb, KT_sb, 0, 0))
            run(proj_qk_block(wq_sb, QT_sb, 0, 0))
            for tt in range(4):
                run(proj_v_block(tt))

            for j in range(NJ):
                for qb in range(NQB):
                    filler_flush_until(need_attention(j, qb))
                    attention(j, qb)
                    if j == NJ - 1:
                        for ob in range(C // 128):
                            filler_add(f"y{qb}.{ob}", proj_y_block(qb, ob))
            # drain the last norm and remaining fillers (tail y projections)
            flush_norm()
            filler_pull(1_000_000_000)

    nc.compile()
    return nc


def _get_nc():
    if "nc" not in _CACHE:
        _CACHE["nc"] = _build_nc()
    return _CACHE["nc"]


def _run(in_maps, trace=False):
    from concourse.bass_utils import run_bass_kernel_spmd

    nc = _get_nc()
    return run_bass_kernel_spmd(nc, in_maps, list(range(N_CORES)), trace=trace)


def _make_in_maps(x, W_Q, W_K, W_V, W_out):
    x = np.asarray(x, dtype=np.float32)
    W_Q = np.asarray(W_Q, dtype=np.float32)
    W_K = np.asarray(W_K, dtype=np.float32)
    W_V = np.asarray(W_V, dtype=np.float32)
    W_out = np.asarray(W_out, dtype=np.float32)

    in_maps = []
    for core in range(N_CORES):
        b, hh = core // 2, core % 2
        sl = slice(hh * DL, (hh + 1) * DL)
        in_maps.append(
            {
                "xt": np.ascontiguousarray(x[b].T).astype(BF16),
                "wqt": np.ascontiguousarray(W_Q[sl, :].T).astype(BF16),
                "wkt": np.ascontiguousarray(W_K[sl, :].T).astype(BF16),
                "wvt": np.ascontiguousarray(W_V[sl, :].T).astype(BF16),
                "wot": np.ascontiguousarray(W_out[:, sl].T).astype(BF16),
            }
        )
    return in_maps


def _assemble(results):
    y = np.empty((B, T, C), dtype=np.float32)
    for b in range(B):
        yT = results[2 * b]["yt"].astype(np.float32) + results[
            2 * b + 1
        ]["yt"].astype(np.float32)
        y[b] = yT.T
    return y


def kernel(x, W_Q, W_K, W_V, W_out):
    res = _run(_make_in_maps(x, W_Q, W_K, W_V, W_out), trace=False)
    return _assemble(res.results)
